# revision 7
# baseline (speedup 1.0000x reference)
"""Trainium2 Bass kernel for a single attention head (nn_AttentionHead).

Problem: B=16, S=2048, W=768, H=64.
  Q = input @ Wq + bq ; K = input @ Wk + bk ; V = input @ Wv + bv
  scores = Q K^T / sqrt(H), key-padding mask, softmax, out = attn @ V.

Sharding: data-parallel over batch across 8 cores (2 samples per core).

v3 design (per core). Two cost facts drive it: TensorE matmul time
depends only on moving columns (contraction depth is free), and ScalarE
exp costs 0.83 ns per score-matrix column. Both scale with the number of
KEY tiles, and masked keys (about half: exp == 0 exactly) contribute
nothing — so the host compacts each sample's keys to the valid subset
(padded to a whole, even number of 128-key tiles; pad keys get a -100
exp bias so they are exactly zero, making compaction bit-equivalent).

  1. Host packs X^T bf16 [B, P, NW, S] for the Q pass, the compacted
     X_kv^T bf16 [B, P, NW, SKV] for the K/V pass, stationaries
     Wq / [Wk|Wv], biases, and the exp bias table (layout prep only).
  2. Q projection (bf16, moving X^T) -> Q^T [64, S]; K/V projection
     (bf16, packed stationary, moving X_kv^T) -> kv [K^T rows 0:64 |
     V^T rows 64:128] over SKV compacted keys. DVE bias-add evacuations.
  3. Scores transposed S^T[k, q] = K^T.T Q^T, plain bf16 matmuls with
     64-deep contraction (cost is moving columns, so depth 64 is free).
  4. exp on ScalarE out of PSUM, scale=1/8 (absorbs 1/sqrt(H); weights
     stay unscaled), bias = -2 margin or -102 for pad keys; the margin
     cancels in the final divide.
  5. V' = [V | ones] rebuilt natural per key tile by TensorE transposes
     of kv rows 64:128 (identity corner at base partition 64). O'^T
     [65, S] accumulated over compacted key tiles in PSUM; row 64 is the
     softmax denominator.
  6. Sample 1's entire prologue is interleaved into sample 0's attention
     loop so TensorE/DVE/DMA work overlaps the exp stream.
  7. Host epilogue: O = O'[:64] / O'[64], transpose to [B, S, H].
"""

import functools

import ml_dtypes
import numpy as np

import concourse.bass as bass
import concourse.bacc as bacc
import concourse.mybir as mybir
import concourse.tile as tile
from concourse.bass_utils import run_bass_kernel_spmd
from concourse.masks import make_identity

F32 = mybir.dt.float32
BF16 = mybir.dt.bfloat16
AF = mybir.ActivationFunctionType
ALU = mybir.AluOpType

P = 128
B_PER_CORE = 2
S = 2048
W = 768
H = 64
NW = W // P      # 6 contraction chunks for the projections
NKT = S // P     # 16 key tiles uncompacted
NQC = S // 512   # 4 query chunks of 512
N_CORES = 8
PAD_BIAS = -100.0   # exp bias for pad keys (exp -> 0 exactly in bf16)
EXP_MARGIN = -2.0   # global exp bias margin (cancels in the divide)
QSCALE = 0.125      # 1/sqrt(H), applied as the exp scale

NP_BF16 = ml_dtypes.bfloat16


def _kv_chunks(skv):
    """PSUM-bank-sized (<=512 col) chunks covering the compacted keys."""
    edges = list(range(0, skv, 512)) + [skv]
    return list(zip(edges[:-1], edges[1:]))


def _emit_q_proj(nc, pools, b, qc):
    wq, bq, xt, qt, pps = (
        pools["wq"], pools["bq"], pools["xt"][b], pools["qt"][b], pools["pps"],
    )
    ps = pps.tile([P, 512], F32, tag="pps", name=f"pq_{b}_{qc}")
    for wc in range(NW):
        nc.tensor.matmul(
            ps[0:H, :],
            wq[:, wc, :],
            xt[:, wc, qc * 512 : (qc + 1) * 512],
            start=(wc == 0),
            stop=(wc == NW - 1),
        )
    nc.vector.tensor_scalar(
        qt[:, qc * 512 : (qc + 1) * 512], ps[0:H, :], bq, None, ALU.add
    )


def _emit_kv_proj(nc, pools, b, c0, c1):
    wkv, bkv, xkv, kv, pps = (
        pools["wkv"], pools["bkv"], pools["xkv"][b], pools["kv"][b], pools["pps"],
    )
    ps = pps.tile([P, 512], F32, tag="pps", name=f"pkv_{b}_{c0}")
    for wc in range(NW):
        nc.tensor.matmul(
            ps[:, 0 : c1 - c0],
            wkv[:, wc, :],
            xkv[:, wc, c0:c1],
            start=(wc == 0),
            stop=(wc == NW - 1),
        )
    nc.vector.tensor_scalar(kv[:, c0:c1], ps[:, 0 : c1 - c0], bkv, None, ALU.add)


def _emit_vtrans(nc, pools, b, j):
    """Transpose kv rows 64:128 (V^T) for key-tile pair (2j, 2j+1) into
    natural bf16 V' tiles."""
    kv, vp, ident, sps = (
        pools["kv"][b], pools["vp"][b], pools["ident"], pools["sps"],
    )
    pst = sps.tile([P, P], BF16, tag="sps", name=f"pvt_{b}_{j}")
    for i in range(2):
        kt = 2 * j + i
        nc.tensor.transpose(
            pst[:, i * H : (i + 1) * H],
            kv[H:P, kt * P : (kt + 1) * P],
            ident[H:P, H:P],
        )
    nc.vector.tensor_copy(
        vp[:, 2 * j : 2 * j + 2, 0:H], pst.rearrange("p (i h) -> p i h", h=H)
    )


def _prologue_stages(nc, pools, b, skv):
    stages = []
    for qc in range(NQC):
        stages.append(functools.partial(_emit_q_proj, nc, pools, b, qc))
    for c0, c1 in _kv_chunks(skv):
        stages.append(functools.partial(_emit_kv_proj, nc, pools, b, c0, c1))
    for j in range(skv // 256):
        stages.append(functools.partial(_emit_vtrans, nc, pools, b, j))
    return stages


def _emit_attention(nc, pools, b, out_e, nkt_kv, interleave=()):
    """Score -> exp -> PV loop for sample b over the compacted key tiles.
    interleave[kt] is a list of thunks emitted at the top of iteration kt
    (the other sample's prologue, to fill engine gaps under the exp
    stream)."""
    qt, kv, vp, ebias = (
        pools["qt"][b], pools["kv"][b], pools["vp"][b], pools["ebias"][b],
    )
    sps_p, ptp, pso_p, oup = pools["sps"], pools["ptp"], pools["pso"], pools["oup"]

    # ones column of V' (row 64 of O'^T = softmax denominator)
    nc.gpsimd.memset(vp[:, :, H : H + 1], 1.0)

    pso = pso_p.tile([H + 1, S], F32, tag="pso", name=f"pso{b}")
    for kt in range(nkt_kv):
        for thunk in (interleave[kt] if kt < len(interleave) else ()):
            thunk()
        pt = ptp.tile([P, S], BF16, tag="pt", name=f"pt_{b}_{kt}")
        for qc in range(NQC):
            sps = sps_p.tile([P, 512], F32, tag="sps", name=f"ss_{b}_{kt}_{qc}")
            nc.tensor.matmul(
                sps,
                kv[0:H, kt * P : (kt + 1) * P],
                qt[:, qc * 512 : (qc + 1) * 512],
                start=True,
                stop=True,
            )
            nc.scalar.activation(
                pt[:, qc * 512 : (qc + 1) * 512],
                sps,
                AF.Exp,
                bias=ebias[:, kt : kt + 1],
                scale=QSCALE,
            )
        for qc in range(NQC):
            nc.tensor.matmul(
                pso[:, qc * 512 : (qc + 1) * 512],
                vp[:, kt, :],
                pt[:, qc * 512 : (qc + 1) * 512],
                start=(kt == 0),
                stop=(kt == nkt_kv - 1),
            )
    ou = oup.tile([H + 1, S], F32, tag="ou", name=f"ou{b}")
    for qc in range(NQC):
        sl = slice(qc * 512, (qc + 1) * 512)
        nc.vector.tensor_copy(ou[:, sl], pso[:, sl])
        nc.sync.dma_start(out=out_e[b, :, sl], in_=ou[:, sl])


def _build(nc, tc, nkt_kv, xt_e, xkv_e, eb_e, wq_e, wkv_e, bq_e, bkv_e, out_e):
    skv = nkt_kv * P
    with (
        tc.tile_pool(name="const", bufs=1) as cpool,
        tc.tile_pool(name="xtp", bufs=2) as xtp,
        tc.tile_pool(name="xkvp", bufs=2) as xkvp,
        tc.tile_pool(name="qtp", bufs=2) as qtp,
        tc.tile_pool(name="kvp", bufs=2) as kvp,
        tc.tile_pool(name="vpp", bufs=2) as vpp,
        tc.tile_pool(name="ptp", bufs=2) as ptp,
        tc.tile_pool(name="oup", bufs=2) as oup,
        tc.tile_pool(name="ebp", bufs=2) as ebp,
        tc.tile_pool(name="sps", bufs=2, space="PSUM") as sps_p,
        tc.tile_pool(name="pps", bufs=2, space="PSUM") as pps,
        tc.tile_pool(name="psop", bufs=1, space="PSUM") as pso_p,
    ):
        ident = cpool.tile([P, P], BF16, name="ident", tag="ident")
        make_identity(nc, ident)
        wq = cpool.tile([P, NW, H], BF16, name="wq", tag="wq")
        wkv = cpool.tile([P, NW, P], BF16, name="wkv", tag="wkv")
        bq = cpool.tile([H, 1], F32, name="bq", tag="bq")
        bkv = cpool.tile([P, 1], F32, name="bkv", tag="bkv")
        nc.gpsimd.dma_start(out=wq, in_=wq_e[:, :, :])
        nc.gpsimd.dma_start(out=wkv, in_=wkv_e[:, :, :])
        nc.gpsimd.dma_start(out=bq, in_=bq_e[:, :])
        nc.gpsimd.dma_start(out=bkv, in_=bkv_e[:, :])

        pools = {
            "ident": ident, "wq": wq, "wkv": wkv, "bq": bq, "bkv": bkv,
            "sps": sps_p, "pps": pps, "pso": pso_p, "ptp": ptp, "oup": oup,
            "xt": [], "xkv": [], "qt": [], "kv": [], "vp": [], "ebias": [],
        }
        for b in range(B_PER_CORE):
            eb = ebp.tile([P, nkt_kv], F32, tag="eb", name=f"eb{b}")
            nc.gpsimd.dma_start(out=eb, in_=eb_e[b])
            pools["ebias"].append(eb)
            pools["xt"].append(xtp.tile([P, NW, S], BF16, tag="xt", name=f"xt{b}"))
            pools["xkv"].append(
                xkvp.tile([P, NW, skv], BF16, tag="xkv", name=f"xkv{b}")
            )
            pools["qt"].append(qtp.tile([H, S], BF16, tag="qt", name=f"qt{b}"))
            pools["kv"].append(kvp.tile([P, skv], BF16, tag="kv", name=f"kv{b}"))
            pools["vp"].append(
                vpp.tile([P, nkt_kv, H + 1], BF16, tag="vp", name=f"vp{b}")
            )

        # input loads, sliced so the first projection groups start early;
        # sample 0 first.
        chunks = _kv_chunks(skv)
        for b in range(B_PER_CORE):
            # first KV chunk before the X^T bulk: the first score matmuls
            # need kv chunk 0 while Q projection is still streaming
            plan = [("xkv", chunks[0])] + [
                ("xt", (qc * 512, (qc + 1) * 512)) for qc in range(NQC)
            ] + [("xkv", c) for c in chunks[1:]]
            eng = nc.sync if b == 0 else nc.gpsimd
            for kind, (c0, c1) in plan:
                dst = pools[kind][b]
                src_e = xt_e if kind == "xt" else xkv_e
                for wc in range(NW):
                    eng.dma_start(
                        out=dst[:, wc, c0:c1],
                        in_=src_e[b, :, wc, c0:c1],
                    )

        # Sample 0: Q projection, KV chunk 0 and vtrans 0 up front; the
        # rest of its V-side work interleaves into the first attention
        # iterations (PV for tile kt only needs V' tile kt).
        s0 = _prologue_stages(nc, pools, 0, skv)
        s1 = _prologue_stages(nc, pools, 1, skv)
        nch = len(_kv_chunks(skv))
        npair = skv // 256
        s0[NQC]()  # KV chunk 0 first: scores(kt 0..3) need it
        for st in s0[:NQC]:  # Q proj x4
            st()
        s0[NQC + nch]()  # vtrans 0
        inter = [[] for _ in range(nkt_kv)]
        for c in range(1, nch):  # KV chunk c at kt=c-1 (covers kts 4c..)
            inter[c - 1].append(s0[NQC + c])
        for j in range(1, npair):  # vtrans j at kt=2j-1 (consumed at 2j)
            inter[2 * j - 1].append(s0[NQC + nch + j])
        # sample 1's full prologue spread over kt=5..nkt_kv-1
        lo = min(5, nkt_kv - 1)
        span = max(nkt_kv - lo, 1)
        for i, st in enumerate(s1):
            inter[lo + (i * span) // len(s1)].append(st)
        _emit_attention(nc, pools, 0, out_e, nkt_kv, interleave=inter)
        _emit_attention(nc, pools, 1, out_e, nkt_kv)


@functools.lru_cache(maxsize=2)
def build_nc(nkt_kv: int) -> bass.Bass:
    skv = nkt_kv * P
    nc = bacc.Bacc()
    xt_e = nc.declare_dram_parameter("xt", [B_PER_CORE, P, NW, S], BF16, isOutput=False)
    xkv_e = nc.declare_dram_parameter(
        "xkv", [B_PER_CORE, P, NW, skv], BF16, isOutput=False
    )
    eb_e = nc.declare_dram_parameter("eb", [B_PER_CORE, P, nkt_kv], F32, isOutput=False)
    wq_e = nc.declare_dram_parameter("wq", [P, NW, H], BF16, isOutput=False)
    wkv_e = nc.declare_dram_parameter("wkv", [P, NW, P], BF16, isOutput=False)
    bq_e = nc.declare_dram_parameter("bq", [H, 1], F32, isOutput=False)
    bkv_e = nc.declare_dram_parameter("bkv", [P, 1], F32, isOutput=False)
    out_e = nc.declare_dram_parameter("out", [B_PER_CORE, H + 1, S], F32, isOutput=True)

    with tile.TileContext(nc, pool_alloc_mode="queue") as tc:
        _build(nc, tc, nkt_kv, xt_e, xkv_e, eb_e, wq_e, wkv_e, bq_e, bkv_e, out_e)
    nc.finalize()
    return nc


def _host_prep(inputs):
    """Pack the full inputs into per-core DRAM layouts (layout/dtype/
    gather prep only; all arithmetic stays on device)."""
    inp = np.asarray(inputs["input"], dtype=np.float32)      # [16, S, W]
    msk = np.asarray(inputs["mask"], dtype=np.int32)         # [16, 1, S]
    B = inp.shape[0]

    # X^T[b, p, wc, s] = X[b, s, wc*128 + p]
    def pack_t(x):
        s = x.shape[1]
        return np.ascontiguousarray(
            x.transpose(0, 2, 1).reshape(B, NW, P, s).transpose(0, 2, 1, 3)
        ).astype(NP_BF16)

    xt = pack_t(inp)

    # compact the keys: per sample gather the valid positions, pad to an
    # even number of whole 128-key tiles (shared across cores: SPMD)
    valid = [np.nonzero(msk[b, 0])[0] for b in range(B)]
    nv_max = max(len(v) for v in valid)
    nkt_kv = min(-(-nv_max // P), NKT)
    nkt_kv = min(nkt_kv + (nkt_kv % 2), NKT)
    skv = nkt_kv * P

    xkv_rows = np.zeros((B, skv, W), dtype=np.float32)
    eb = np.full((B, skv), PAD_BIAS, dtype=np.float32)
    for b in range(B):
        v = valid[b][:skv]
        xkv_rows[b, : len(v)] = inp[b, v]
        eb[b, : len(v)] = 0.0
    xkv = pack_t(xkv_rows)
    eb = (eb + EXP_MARGIN).reshape(B, nkt_kv, P).transpose(0, 2, 1)
    eb = np.ascontiguousarray(eb)

    wq_in = np.asarray(inputs["Wq"], dtype=np.float32)
    wk = np.asarray(inputs["Wk"], dtype=np.float32)
    wv = np.asarray(inputs["Wv"], dtype=np.float32)
    wq = np.ascontiguousarray(wq_in.reshape(NW, P, H).transpose(1, 0, 2)).astype(
        NP_BF16
    )
    wkv = np.concatenate([wk, wv], axis=1).reshape(NW, P, 2 * H).transpose(1, 0, 2)
    wkv = np.ascontiguousarray(wkv).astype(NP_BF16)

    bq = np.asarray(inputs["bq"], dtype=np.float32)[:, None]
    bkv = np.concatenate(
        [np.asarray(inputs["bk"]), np.asarray(inputs["bv"])]
    ).astype(np.float32)[:, None]
    return nkt_kv, xt, xkv, eb, wq, wkv, bq, bkv


def run(inputs, trace=False, **kwargs):
    nkt_kv, xt, xkv, eb, wq, wkv, bq, bkv = _host_prep(inputs)
    nc = build_nc(nkt_kv)
    in_maps = []
    for c in range(N_CORES):
        sl = slice(B_PER_CORE * c, B_PER_CORE * (c + 1))
        in_maps.append({
            "xt": xt[sl], "xkv": xkv[sl], "eb": eb[sl],
            "wq": wq, "wkv": wkv, "bq": bq, "bkv": bkv,
        })
    res = run_bass_kernel_spmd(nc, in_maps, list(range(N_CORES)), trace=trace, **kwargs)
    outs = np.concatenate(
        [res.results[i]["out"] for i in range(N_CORES)], axis=0
    )  # [16, 65, 2048]
    o = outs[:, :H, :] / outs[:, H : H + 1, :]
    return np.ascontiguousarray(o.transpose(0, 2, 1)).astype(np.float32), res


def kernel(**inputs):
    out, _ = run(inputs, trace=False)
    return out


# revision 8
# speedup vs baseline: 1.1631x; 1.1631x over previous
"""Trainium2 Bass kernel for a single attention head (nn_AttentionHead).

Problem: B=16, S=2048, W=768, H=64.
  Q = input @ Wq + bq ; K = input @ Wk + bk ; V = input @ Wv + bv
  scores = Q K^T / sqrt(H), key-padding mask, softmax, out = attn @ V.

Sharding: data-parallel over batch across 8 cores (2 samples per core).

v3 design (per core). Two cost facts drive it: TensorE matmul time
depends only on moving columns (contraction depth is free), and ScalarE
exp costs 0.83 ns per score-matrix column. Both scale with the number of
KEY tiles, and masked keys (about half: exp == 0 exactly) contribute
nothing — so the host compacts each sample's keys to the valid subset
(padded to a whole, even number of 128-key tiles; pad keys get a -100
exp bias so they are exactly zero, making compaction bit-equivalent).

  1. Host packs X^T bf16 [B, P, NW, S] for the Q pass, the compacted
     X_kv^T bf16 [B, P, NW, SKV] for the K/V pass, stationaries
     Wq / [Wk|Wv], biases, and the exp bias table (layout prep only).
  2. Q projection (bf16, moving X^T) -> Q^T [64, S]; K/V projection
     (bf16, packed stationary, moving X_kv^T) -> kv [K^T rows 0:64 |
     V^T rows 64:128] over SKV compacted keys. DVE bias-add evacuations.
  3. Scores transposed S^T[k, q] = K^T.T Q^T, plain bf16 matmuls with
     64-deep contraction (cost is moving columns, so depth 64 is free).
  4. exp on ScalarE out of PSUM, scale=1/8 (absorbs 1/sqrt(H); weights
     stay unscaled), bias = -2 margin or -102 for pad keys; the margin
     cancels in the final divide.
  5. V' = [V | ones] rebuilt natural per key tile by TensorE transposes
     of kv rows 64:128 (identity corner at base partition 64). O'^T
     [65, S] accumulated over compacted key tiles in PSUM; row 64 is the
     softmax denominator.
  6. Sample 1's entire prologue is interleaved into sample 0's attention
     loop so TensorE/DVE/DMA work overlaps the exp stream.
  7. Host epilogue: O = O'[:64] / O'[64], transpose to [B, S, H].
"""

import functools

import ml_dtypes
import numpy as np

import concourse.bass as bass
import concourse.bacc as bacc
import concourse.mybir as mybir
import concourse.tile as tile
from concourse.bass_utils import run_bass_kernel_spmd
from concourse.masks import make_identity

F32 = mybir.dt.float32
BF16 = mybir.dt.bfloat16
AF = mybir.ActivationFunctionType
ALU = mybir.AluOpType

P = 128
B_PER_CORE = 2
S = 2048
W = 768
H = 64
NW = W // P      # 6 contraction chunks for the projections
NKT = S // P     # 16 key tiles uncompacted
NQC = S // 512   # 4 query chunks of 512
N_CORES = 8
PAD_BIAS = -100.0   # exp bias for pad keys (exp -> 0 exactly in bf16)
EXP_MARGIN = -2.0   # global exp bias margin (cancels in the divide)
QSCALE = 0.125      # 1/sqrt(H), applied as the exp scale

NP_BF16 = ml_dtypes.bfloat16


def _kv_chunks(skv):
    """PSUM-bank-sized (<=512 col) chunks covering the compacted keys."""
    edges = list(range(0, skv, 512)) + [skv]
    return list(zip(edges[:-1], edges[1:]))


def _emit_q_proj(nc, pools, b, qc):
    wq, bq, xt, qt, pps = (
        pools["wq"], pools["bq"], pools["xt"][b], pools["qt"][b], pools["pps"],
    )
    ps = pps.tile([P, 512], F32, tag="pps", name=f"pq_{b}_{qc}")
    for wc in range(NW):
        nc.tensor.matmul(
            ps[0:H, :],
            wq[:, wc, :],
            xt[:, wc, qc * 512 : (qc + 1) * 512],
            start=(wc == 0),
            stop=(wc == NW - 1),
        )
    nc.vector.tensor_scalar(
        qt[:, qc * 512 : (qc + 1) * 512], ps[0:H, :], bq, None, ALU.add
    )


def _emit_kv_proj(nc, pools, b, c0, c1):
    wkv, bkv, xkv, kv, pps = (
        pools["wkv"], pools["bkv"], pools["xkv"][b], pools["kv"][b], pools["pps"],
    )
    ps = pps.tile([P, 512], F32, tag="pps", name=f"pkv_{b}_{c0}")
    for wc in range(NW):
        nc.tensor.matmul(
            ps[:, 0 : c1 - c0],
            wkv[:, wc, :],
            xkv[:, wc, c0:c1],
            start=(wc == 0),
            stop=(wc == NW - 1),
        )
    nc.vector.tensor_scalar(kv[:, c0:c1], ps[:, 0 : c1 - c0], bkv, None, ALU.add)


def _emit_vtrans(nc, pools, b, j):
    """Transpose kv rows 64:128 (V^T) for key-tile pair (2j, 2j+1) into
    natural bf16 V' tiles."""
    kv, vp, ident, sps = (
        pools["kv"][b], pools["vp"][b], pools["ident"], pools["sps"],
    )
    pst = sps.tile([P, P], BF16, tag="sps", name=f"pvt_{b}_{j}")
    for i in range(2):
        kt = 2 * j + i
        nc.tensor.transpose(
            pst[:, i * H : (i + 1) * H],
            kv[H:P, kt * P : (kt + 1) * P],
            ident[H:P, H:P],
        )
    nc.vector.tensor_copy(
        vp[:, 2 * j : 2 * j + 2, 0:H], pst.rearrange("p (i h) -> p i h", h=H)
    )


def _prologue_stages(nc, pools, b, skv):
    stages = []
    for qc in range(NQC):
        stages.append(functools.partial(_emit_q_proj, nc, pools, b, qc))
    for c0, c1 in _kv_chunks(skv):
        stages.append(functools.partial(_emit_kv_proj, nc, pools, b, c0, c1))
    for j in range(skv // 256):
        stages.append(functools.partial(_emit_vtrans, nc, pools, b, j))
    return stages


def _emit_attention(nc, pools, b, out_e, nkt_kv, interleave=()):
    """Score -> exp -> PV loop for sample b over the compacted key tiles.
    interleave[kt] is a list of thunks emitted at the top of iteration kt
    (the other sample's prologue, to fill engine gaps under the exp
    stream)."""
    qt, kv, vp, ebias = (
        pools["qt"][b], pools["kv"][b], pools["vp"][b], pools["ebias"][b],
    )
    sps_p, ptp, pso_p, oup = pools["sps"], pools["ptp"], pools["pso"], pools["oup"]

    # ones column of V' (row 64 of O'^T = softmax denominator)
    nc.vector.memset(vp[:, :, H : H + 1], 1.0)

    pso = pso_p.tile([H + 1, S], F32, tag="pso", name=f"pso{b}")
    for kt in range(nkt_kv):
        for thunk in (interleave[kt] if kt < len(interleave) else ()):
            thunk()
        pt = ptp.tile([P, S], BF16, tag="pt", name=f"pt_{b}_{kt}")
        for qc in range(NQC):
            sps = sps_p.tile([P, 512], F32, tag="sps", name=f"ss_{b}_{kt}_{qc}")
            nc.tensor.matmul(
                sps,
                kv[0:H, kt * P : (kt + 1) * P],
                qt[:, qc * 512 : (qc + 1) * 512],
                start=True,
                stop=True,
            )
            nc.scalar.activation(
                pt[:, qc * 512 : (qc + 1) * 512],
                sps,
                AF.Exp,
                bias=ebias[:, kt : kt + 1],
                scale=QSCALE,
            )
        for qc in range(NQC):
            nc.tensor.matmul(
                pso[:, qc * 512 : (qc + 1) * 512],
                vp[:, kt, :],
                pt[:, qc * 512 : (qc + 1) * 512],
                start=(kt == 0),
                stop=(kt == nkt_kv - 1),
            )
    ou = oup.tile([H + 1, S], F32, tag="ou", name=f"ou{b}")
    for qc in range(NQC):
        sl = slice(qc * 512, (qc + 1) * 512)
        nc.vector.tensor_copy(ou[:, sl], pso[:, sl])
        nc.sync.dma_start(out=out_e[b, :, sl], in_=ou[:, sl])


def _build(nc, tc, nkt_kv, xt_e, xkv_e, eb_e, wq_e, wkv_e, bq_e, bkv_e, out_e):
    skv = nkt_kv * P
    with (
        tc.tile_pool(name="const", bufs=1) as cpool,
        tc.tile_pool(name="xtp", bufs=2) as xtp,
        tc.tile_pool(name="xkvp", bufs=2) as xkvp,
        tc.tile_pool(name="qtp", bufs=2) as qtp,
        tc.tile_pool(name="kvp", bufs=2) as kvp,
        tc.tile_pool(name="vpp", bufs=2) as vpp,
        tc.tile_pool(name="ptp", bufs=2) as ptp,
        tc.tile_pool(name="oup", bufs=2) as oup,
        tc.tile_pool(name="ebp", bufs=2) as ebp,
        tc.tile_pool(name="sps", bufs=3, space="PSUM") as sps_p,
        tc.tile_pool(name="pps", bufs=1, space="PSUM") as pps,
        tc.tile_pool(name="psop", bufs=1, space="PSUM") as pso_p,
    ):
        ident = cpool.tile([P, P], BF16, name="ident", tag="ident")
        make_identity(nc, ident)
        wq = cpool.tile([P, NW, H], BF16, name="wq", tag="wq")
        wkv = cpool.tile([P, NW, P], BF16, name="wkv", tag="wkv")
        bq = cpool.tile([H, 1], F32, name="bq", tag="bq")
        bkv = cpool.tile([P, 1], F32, name="bkv", tag="bkv")
        nc.gpsimd.dma_start(out=wq, in_=wq_e[:, :, :])
        nc.gpsimd.dma_start(out=wkv, in_=wkv_e[:, :, :])
        nc.gpsimd.dma_start(out=bq, in_=bq_e[:, :])
        nc.gpsimd.dma_start(out=bkv, in_=bkv_e[:, :])

        pools = {
            "ident": ident, "wq": wq, "wkv": wkv, "bq": bq, "bkv": bkv,
            "sps": sps_p, "pps": pps, "pso": pso_p, "ptp": ptp, "oup": oup,
            "xt": [], "xkv": [], "qt": [], "kv": [], "vp": [], "ebias": [],
        }
        for b in range(B_PER_CORE):
            eb = ebp.tile([P, nkt_kv], F32, tag="eb", name=f"eb{b}")
            nc.gpsimd.dma_start(out=eb, in_=eb_e[b])
            pools["ebias"].append(eb)
            pools["xt"].append(xtp.tile([P, NW, S], BF16, tag="xt", name=f"xt{b}"))
            pools["xkv"].append(
                xkvp.tile([P, NW, skv], BF16, tag="xkv", name=f"xkv{b}")
            )
            pools["qt"].append(qtp.tile([H, S], BF16, tag="qt", name=f"qt{b}"))
            pools["kv"].append(kvp.tile([P, skv], BF16, tag="kv", name=f"kv{b}"))
            pools["vp"].append(
                vpp.tile([P, nkt_kv, H + 1], BF16, tag="vp", name=f"vp{b}")
            )

        # input loads, sliced so the first projection groups start early;
        # sample 0 first.
        chunks = _kv_chunks(skv)
        for b in range(B_PER_CORE):
            # first KV chunk before the X^T bulk: the first score matmuls
            # need kv chunk 0 while Q projection is still streaming
            plan = [("xkv", chunks[0])] + [
                ("xt", (qc * 512, (qc + 1) * 512)) for qc in range(NQC)
            ] + [("xkv", c) for c in chunks[1:]]
            eng = nc.sync if b == 0 else nc.gpsimd
            for kind, (c0, c1) in plan:
                dst = pools[kind][b]
                src_e = xt_e if kind == "xt" else xkv_e
                for wc in range(NW):
                    eng.dma_start(
                        out=dst[:, wc, c0:c1],
                        in_=src_e[b, :, wc, c0:c1],
                    )

        # Sample 0: Q projection, KV chunk 0 and vtrans 0 up front; the
        # rest of its V-side work interleaves into the first attention
        # iterations (PV for tile kt only needs V' tile kt).
        s0 = _prologue_stages(nc, pools, 0, skv)
        s1 = _prologue_stages(nc, pools, 1, skv)
        nch = len(_kv_chunks(skv))
        npair = skv // 256
        s0[NQC]()  # KV chunk 0 first: scores(kt 0..3) need it
        for st in s0[:NQC]:  # Q proj x4
            st()
        s0[NQC + nch]()  # vtrans 0
        inter = [[] for _ in range(nkt_kv)]
        for c in range(1, nch):  # KV chunk c at kt=c-1 (covers kts 4c..)
            inter[c - 1].append(s0[NQC + c])
        for j in range(1, npair):  # vtrans j at kt=2j-1 (consumed at 2j)
            inter[2 * j - 1].append(s0[NQC + nch + j])
        # sample 1's full prologue spread over kt=4..nkt_kv-1
        lo = min(4, nkt_kv - 1)
        span = max(nkt_kv - lo, 1)
        for i, st in enumerate(s1):
            inter[lo + (i * span) // len(s1)].append(st)
        _emit_attention(nc, pools, 0, out_e, nkt_kv, interleave=inter)
        _emit_attention(nc, pools, 1, out_e, nkt_kv)


@functools.lru_cache(maxsize=2)
def build_nc(nkt_kv: int) -> bass.Bass:
    skv = nkt_kv * P
    nc = bacc.Bacc()
    xt_e = nc.declare_dram_parameter("xt", [B_PER_CORE, P, NW, S], BF16, isOutput=False)
    xkv_e = nc.declare_dram_parameter(
        "xkv", [B_PER_CORE, P, NW, skv], BF16, isOutput=False
    )
    eb_e = nc.declare_dram_parameter("eb", [B_PER_CORE, P, nkt_kv], F32, isOutput=False)
    wq_e = nc.declare_dram_parameter("wq", [P, NW, H], BF16, isOutput=False)
    wkv_e = nc.declare_dram_parameter("wkv", [P, NW, P], BF16, isOutput=False)
    bq_e = nc.declare_dram_parameter("bq", [H, 1], F32, isOutput=False)
    bkv_e = nc.declare_dram_parameter("bkv", [P, 1], F32, isOutput=False)
    out_e = nc.declare_dram_parameter("out", [B_PER_CORE, H + 1, S], F32, isOutput=True)

    with tile.TileContext(nc, pool_alloc_mode="queue") as tc:
        _build(nc, tc, nkt_kv, xt_e, xkv_e, eb_e, wq_e, wkv_e, bq_e, bkv_e, out_e)
    nc.finalize()
    return nc


def _host_prep(inputs):
    """Pack the full inputs into per-core DRAM layouts (layout/dtype/
    gather prep only; all arithmetic stays on device)."""
    inp = np.asarray(inputs["input"], dtype=np.float32)      # [16, S, W]
    msk = np.asarray(inputs["mask"], dtype=np.int32)         # [16, 1, S]
    B = inp.shape[0]

    # X^T[b, p, wc, s] = X[b, s, wc*128 + p]
    def pack_t(x):
        s = x.shape[1]
        return np.ascontiguousarray(
            x.transpose(0, 2, 1).reshape(B, NW, P, s).transpose(0, 2, 1, 3)
        ).astype(NP_BF16)

    xt = pack_t(inp)

    # compact the keys: per sample gather the valid positions, pad to an
    # even number of whole 128-key tiles (shared across cores: SPMD)
    valid = [np.nonzero(msk[b, 0])[0] for b in range(B)]
    nv_max = max(len(v) for v in valid)
    nkt_kv = min(-(-nv_max // P), NKT)
    nkt_kv = min(nkt_kv + (nkt_kv % 2), NKT)
    skv = nkt_kv * P

    xkv_rows = np.zeros((B, skv, W), dtype=np.float32)
    eb = np.full((B, skv), PAD_BIAS, dtype=np.float32)
    for b in range(B):
        v = valid[b][:skv]
        xkv_rows[b, : len(v)] = inp[b, v]
        eb[b, : len(v)] = 0.0
    xkv = pack_t(xkv_rows)
    eb = (eb + EXP_MARGIN).reshape(B, nkt_kv, P).transpose(0, 2, 1)
    eb = np.ascontiguousarray(eb)

    wq_in = np.asarray(inputs["Wq"], dtype=np.float32)
    wk = np.asarray(inputs["Wk"], dtype=np.float32)
    wv = np.asarray(inputs["Wv"], dtype=np.float32)
    wq = np.ascontiguousarray(wq_in.reshape(NW, P, H).transpose(1, 0, 2)).astype(
        NP_BF16
    )
    wkv = np.concatenate([wk, wv], axis=1).reshape(NW, P, 2 * H).transpose(1, 0, 2)
    wkv = np.ascontiguousarray(wkv).astype(NP_BF16)

    bq = np.asarray(inputs["bq"], dtype=np.float32)[:, None]
    bkv = np.concatenate(
        [np.asarray(inputs["bk"]), np.asarray(inputs["bv"])]
    ).astype(np.float32)[:, None]
    return nkt_kv, xt, xkv, eb, wq, wkv, bq, bkv


def run(inputs, trace=False, **kwargs):
    nkt_kv, xt, xkv, eb, wq, wkv, bq, bkv = _host_prep(inputs)
    nc = build_nc(nkt_kv)
    in_maps = []
    for c in range(N_CORES):
        sl = slice(B_PER_CORE * c, B_PER_CORE * (c + 1))
        in_maps.append({
            "xt": xt[sl], "xkv": xkv[sl], "eb": eb[sl],
            "wq": wq, "wkv": wkv, "bq": bq, "bkv": bkv,
        })
    res = run_bass_kernel_spmd(nc, in_maps, list(range(N_CORES)), trace=trace, **kwargs)
    outs = np.concatenate(
        [res.results[i]["out"] for i in range(N_CORES)], axis=0
    )  # [16, 65, 2048]
    o = outs[:, :H, :] / outs[:, H : H + 1, :]
    return np.ascontiguousarray(o.transpose(0, 2, 1)).astype(np.float32), res


def kernel(**inputs):
    out, _ = run(inputs, trace=False)
    return out
